# revision 1
# baseline (speedup 1.0000x reference)
"""Multi-head dense attention (no softmax) on 8 Trainium2 NeuronCores.

Math (per batch b, head h with head_dim d=64):
    q   = x @ W^T                      # [S, H] projection
    out_h = (q_h x_h^T) x_h            # naive: O(S^2 d) with an SxS temp
          = q_h (x_h^T x_h)            # reassociated: Gram matrix G_h [d, d]
The reassociation is exact (same sum, different order) and collapses the
FLOPs ~5x while removing the SxS intermediate entirely.

Sharding: core c handles batch b = c//2 and head-group hg = c%2 (8 heads,
512 output columns). Cores are fully independent (no collectives).

Device layout per core (all inputs fp16; W is pre-scaled by 1024 on the
host so its sigma~9e-5 entries clear fp16's subnormal cutoff; the Gram
tile copy multiplies by 1/1024 to undo it):
    xT  [1024, 2048]  x[b] transposed (host-prepped)  - projection operands
    xn  [2048, 512]   x[b] natural, this head-group's columns - Gram operands
    wT  [1024, 512]   1024 * W rows of this head-group, transposed (k-major)
    outT [512, 2048]  output transposed (fp16); host transposes/upcasts back

Schedule (fp16 matmuls: 1 cycle/row at N=512, fp32 PSUM; the kernel is
co-limited by PE throughput and the ~300 GB/s per-core DMA fabric, so
the projection's k-contraction is split in half around the DMA stream):
    1. halfA: qA' = (1024 W) x^T over k-tiles 0-1, while those tiles land.
    2. Gram:  per head-pair p, psum += xn_p^T xn_p over 16 s-tiles; the two
       64x64 diagonal blocks are copied (scaled 1/1024) into a zeroed
       block-diagonal fp16 lhsT tile. Runs while xn + xT 2-7 stream in.
    3. halfB: k-tiles 2-7; the PSUM->SBUF drain is a tensor_add that folds
       qA' in (qT' = psum + qA'), and after each m-tile p the output stage
       emits outT_p = Gbd_p^T qT'_p as one N=512 matmul per s-chunk, then
       copies PSUM->SBUF (fp16) and stores on the sync/gpsimd rings.
wT is host-blocked m-tile-major so every weight descriptor reads 2KB
contiguous lines.
All input descriptors ride the Activation engine's DMA ring in exact
consumption order; stores alternate the Sync and GpSimd rings.
"""

import numpy as np

B, S, H = 4, 2048, 1024
N_HEADS = 16
HD = H // N_HEADS  # 64
N_CORES = 8
MG = H // 2        # 512 output columns per core
P = 128
KT = H // P        # 8 k-tiles
ST = S // P        # 16 s-tiles
MT = MG // P       # 4 m-tiles == head pairs
SC = S // 512      # 4 s-chunks
W_SCALE = 1024.0
KT_A = 2             # k-tiles in the first projection half

_NC_CACHE = {}


def _build_nc():
    import concourse.mybir as mybir
    from concourse import bacc
    from concourse.tile import TileContext

    f32 = mybir.dt.float32
    f16 = mybir.dt.float16

    nc = bacc.Bacc()
    xT_d = nc.declare_dram_parameter("xT", [H, S], f16, isOutput=False)
    xn_d = nc.declare_dram_parameter("xn", [S, MG], f16, isOutput=False)
    wT_d = nc.declare_dram_parameter("wT", [MT * P, KT * P], f16, isOutput=False)
    outT_d = nc.declare_dram_parameter("outT", [MG, S], f16, isOutput=True)

    xT_t = xT_d.rearrange("(kt p) s -> p kt s", p=P)   # [128, 8, 2048]
    xn_t = xn_d.rearrange("(st p) m -> p st m", p=P)   # [128, 16, 512]
    wT_t = wT_d.rearrange("(mt p) (kt m) -> p mt kt m", p=P, m=P)  # [128, 4, 8, 128]

    with TileContext(nc) as tc:
        with (
            tc.tile_pool(name="big", bufs=1) as big,
            tc.tile_pool(name="gp", bufs=1) as gpool,
            tc.tile_pool(name="stage", bufs=4) as stage,
            tc.tile_pool(name="ps_q", bufs=1, space="PSUM") as ps_q,
        ):
            xT_sb = big.tile([P, KT, S], f16, tag="xT")
            xn_sb = big.tile([P, ST, MG], f16, tag="xn")
            wT_sb = big.tile([P, MT, KT, P], f16, tag="wT")
            qA_sb = big.tile([P, MT, S], f16, tag="qA")
            qB_sb = big.tile([P, MT, S], f16, tag="qB")

            # Single Activation-engine ring, descriptors in consumption
            # order: wT m-tile 0, xT k-tiles 0-1 (first projection half),
            # remaining wT, xn (Gram runs in the middle), xT k-tiles 2-7
            # (second half). Output stores alternate Sync/GpSimd rings.
            nc.scalar.dma_start(out=wT_sb[:, 0], in_=wT_t[:, 0])
            nc.scalar.dma_start(out=xT_sb[:, 0], in_=xT_t[:, 0])
            nc.scalar.dma_start(out=xT_sb[:, 1], in_=xT_t[:, 1])
            nc.scalar.dma_start(out=wT_sb[:, 1:], in_=wT_t[:, 1:])
            nc.scalar.dma_start(out=xn_sb[:, :8], in_=xn_t[:, :8])
            nc.scalar.dma_start(out=xn_sb[:, 8:], in_=xn_t[:, 8:])
            for kt in range(KT_A, KT):
                nc.scalar.dma_start(out=xT_sb[:, kt], in_=xT_t[:, kt])

            def proj_half(kts, q_sb, phase):
                for mt in range(MT):
                    psqs = [
                        ps_q.tile(
                            [P, 512], f32, tag=f"psq{sc}", name=f"psq{phase}_{mt}_{sc}"
                        )
                        for sc in range(SC)
                    ]
                    for n, kt in enumerate(kts):
                        for sc in range(SC):
                            nc.tensor.matmul(
                                psqs[sc],
                                lhsT=wT_sb[:, mt, kt],
                                rhs=xT_sb[:, kt, sc * 512:(sc + 1) * 512],
                                start=(n == 0),
                                stop=(n == len(kts) - 1),
                            )
                    for sc in range(SC):
                        nc.vector.tensor_copy(
                            out=q_sb[:, mt, sc * 512:(sc + 1) * 512], in_=psqs[sc]
                        )

            # ---- First projection half: only needs wT + xT k-tiles 0-1.
            proj_half(range(KT_A), qA_sb, "A")

            # ---- Gram stage: needs xn, which lands mid-stream.
            gbd = []
            with tc.tile_pool(name="ps_g", bufs=2, space="PSUM") as ps_g:
                for p_i in range(MT):
                    psg = ps_g.tile([P, P], f32, tag="psg", name=f"psg{p_i}")
                    xp = xn_sb[:, :, p_i * P:(p_i + 1) * P]
                    for i in range(ST):
                        nc.tensor.matmul(
                            psg,
                            lhsT=xp[:, i],
                            rhs=xp[:, i],
                            start=(i == 0),
                            stop=(i == ST - 1),
                        )
                    g = gpool.tile([P, P], f16, tag=f"g{p_i}", name=f"g{p_i}")
                    nc.vector.memset(g, 0.0)
                    nc.vector.tensor_scalar_mul(
                        out=g[0:HD, 0:HD], in0=psg[0:HD, 0:HD], scalar1=1.0 / W_SCALE
                    )
                    nc.vector.tensor_scalar_mul(
                        out=g[HD:P, HD:P], in0=psg[HD:P, HD:P], scalar1=1.0 / W_SCALE
                    )
                    gbd.append(g)

            with tc.tile_pool(name="ps_o", bufs=4, space="PSUM") as ps_o:
                # ---- Second projection half (xT k-tiles 4-7), with pair p's
                # output stage emitted one m-tile behind: out = G qA + G qB
                # accumulated in PSUM (linearity), so the halves never need to
                # be added explicitly.
                def emit_out(p_i):
                    for sc in range(SC):
                        pso = ps_o.tile([P, 512], f32, tag="pso", name=f"pso{p_i}_{sc}")
                        nc.tensor.matmul(
                            pso,
                            lhsT=gbd[p_i],
                            rhs=qB_sb[:, p_i, sc * 512:(sc + 1) * 512],
                            start=True,
                            stop=True,
                        )
                        ot = stage.tile([P, 512], f16, tag="ot", name=f"ot{p_i}_{sc}")
                        nc.vector.tensor_copy(out=ot, in_=pso)
                        st_eng = nc.sync if sc % 2 == 0 else nc.gpsimd
                        st_eng.dma_start(
                            out=outT_d[p_i * P:(p_i + 1) * P, sc * 512:(sc + 1) * 512],
                            in_=ot,
                        )

                for mt in range(MT):
                    psqs = [
                        ps_q.tile([P, 512], f32, tag=f"psq{sc}", name=f"psqB{mt}_{sc}")
                        for sc in range(SC)
                    ]
                    kts_b = range(KT_A, KT)
                    for n, kt in enumerate(kts_b):
                        for sc in range(SC):
                            nc.tensor.matmul(
                                psqs[sc],
                                lhsT=wT_sb[:, mt, kt],
                                rhs=xT_sb[:, kt, sc * 512:(sc + 1) * 512],
                                start=(n == 0),
                                stop=(n == KT // 2 - 1),
                            )
                    for sc in range(SC):
                        nc.vector.tensor_add(
                            out=qB_sb[:, mt, sc * 512:(sc + 1) * 512],
                            in0=psqs[sc],
                            in1=qA_sb[:, mt, sc * 512:(sc + 1) * 512],
                        )
                    emit_out(mt)
    nc.compile()
    return nc


def _get_nc():
    if "nc" not in _NC_CACHE:
        _NC_CACHE["nc"] = _build_nc()
    return _NC_CACHE["nc"]


def make_in_maps(hidden_states, queries_weight):
    hs = np.ascontiguousarray(np.asarray(hidden_states, dtype=np.float32))
    w = np.ascontiguousarray(np.asarray(queries_weight, dtype=np.float32))
    in_maps = []
    for c in range(N_CORES):
        b, hg = divmod(c, 2)
        xb = hs[b]
        in_maps.append({
            "xT": np.ascontiguousarray(xb.T).astype(np.float16),
            "xn": np.ascontiguousarray(xb[:, hg * MG:(hg + 1) * MG]).astype(
                np.float16
            ),
            "wT": np.ascontiguousarray(
                (w[hg * MG:(hg + 1) * MG, :].T * W_SCALE)
                .reshape(KT, P, MT, P)
                .transpose(2, 1, 0, 3)
                .reshape(MT * P, KT * P)
            ).astype(np.float16),
        })
    return in_maps


def assemble_output(results):
    out = np.empty((B, S, H), dtype=np.float32)
    for c in range(N_CORES):
        b, hg = divmod(c, 2)
        out[b, :, hg * MG:(hg + 1) * MG] = results[c]["outT"].T.astype(np.float32)
    return out


def kernel(hidden_states, queries_weight):
    from concourse.bass_utils import run_bass_kernel_spmd

    in_maps = make_in_maps(hidden_states, queries_weight)
    res = run_bass_kernel_spmd(
        _get_nc(), in_maps, core_ids=list(range(N_CORES))
    ).results
    return assemble_output(res)


if __name__ == "__main__":
    x = np.random.randn(B, S, H).astype(np.float32)
    w = np.random.randn(H, H).astype(np.float32) * 1e-4
    out = kernel(x, w)
    print(out.shape, out.dtype)



# revision 2
# speedup vs baseline: 1.1569x; 1.1569x over previous
"""Multi-head dense attention (no softmax) on 8 Trainium2 NeuronCores.

Math (per batch b, head h with head_dim d=64):
    q   = x @ W^T                      # [S, H] projection
    out_h = (q_h x_h^T) x_h            # naive: O(S^2 d) with an SxS temp
          = q_h (x_h^T x_h)            # reassociated: Gram matrix G_h [d, d]
The reassociation is exact (same sum, different order) and collapses the
FLOPs ~5x while removing the SxS intermediate entirely.

Sharding: core c handles batch b = c//2 and head-group hg = c%2 (8 heads,
512 output columns). Cores are fully independent (no collectives).

v2 restructure vs the first working kernel:
  - Projection accumulates all 8 k-tiles in one PSUM group per m-tile
    (no qA/qB halves -> no 16 tensor_tensor adds on DVE).
  - All input DMAs ride the SP (sync) queue in exact consumption order;
    stores ride GpSimd; DVE/ACT only do PSUM drains.
  - A short warmup matmul burst keeps the PE clock ramped while the
    first xT k-tiles stream in (HAM throttling).
  - Gram runs between projection m-tiles 1 and 2 (xn has landed by
    then); per-pair output matmuls are interleaved after gram so their
    PSUM drains overlap remaining projection work.
"""

import numpy as np

B, S, H = 4, 2048, 1024
N_HEADS = 16
HD = H // N_HEADS  # 64
N_CORES = 8
MG = H // 2        # 512 output columns per core
P = 128
KT = H // P        # 8 k-tiles
ST = S // P        # 16 s-tiles
MT = MG // P       # 4 m-tiles == head pairs
SC = S // 512      # 4 s-chunks
W_SCALE = 1024.0
N_WARM = 8

_NC_CACHE = {}


def _build_nc():
    import concourse.mybir as mybir
    from concourse import bacc
    from concourse.tile import TileContext

    f32 = mybir.dt.float32
    f16 = mybir.dt.float16

    nc = bacc.Bacc()
    xT_d = nc.declare_dram_parameter("xT", [H, S], f16, isOutput=False)
    xn_d = nc.declare_dram_parameter("xn", [S, MG], f16, isOutput=False)
    wT_d = nc.declare_dram_parameter("wT", [MT * P, KT * P], f16, isOutput=False)
    outT_d = nc.declare_dram_parameter("outT", [MG, S], f16, isOutput=True)

    xT_t = xT_d.rearrange("(kt p) s -> p kt s", p=P)   # [128, 8, 2048]
    xn_t = xn_d.rearrange("(st p) m -> p st m", p=P)   # [128, 16, 512]
    wT_t = wT_d.rearrange("(mt p) (kt m) -> p mt kt m", p=P, m=P)  # [128, 4, 8, 128]

    with TileContext(nc) as tc:
        with (
            tc.tile_pool(name="big", bufs=1) as big,
            tc.tile_pool(name="gp", bufs=1) as gpool,
            tc.tile_pool(name="stage", bufs=8) as stage,
            tc.tile_pool(name="ps_q", bufs=1, space="PSUM") as ps_q,
            tc.tile_pool(name="ps_g", bufs=2, space="PSUM") as ps_g,
            tc.tile_pool(name="ps_o", bufs=2, space="PSUM") as ps_o,
        ):
            xT_sb = big.tile([P, KT, S], f16, tag="xT")
            xn_sb = big.tile([P, ST, MG], f16, tag="xn")
            wT_sb = big.tile([P, MT, KT, P], f16, tag="wT")
            qT_sb = big.tile([P, MT, S], f16, tag="qT")
            warm_sb = big.tile([P, 512], f16, tag="warm")

            # Early engine-local setup (Pool queue): warmup operand and
            # zeroed block-diagonal G holders.
            nc.gpsimd.memset(warm_sb, 0.0)
            gbd = []
            for p_i in range(MT):
                g = gpool.tile([P, P], f16, tag=f"g{p_i}", name=f"g{p_i}")
                nc.gpsimd.memset(g, 0.0)
                gbd.append(g)

            # All input loads on the SP queue, in consumption order.
            nc.sync.dma_start(out=wT_sb[:, 0], in_=wT_t[:, 0])
            for kt in range(KT):
                nc.sync.dma_start(out=xT_sb[:, kt], in_=xT_t[:, kt])
            nc.sync.dma_start(out=wT_sb[:, 1:], in_=wT_t[:, 1:])
            nc.sync.dma_start(out=xn_sb[:, :8], in_=xn_t[:, :8])
            nc.sync.dma_start(out=xn_sb[:, 8:], in_=xn_t[:, 8:])

            def emit_gram():
                for p_i in range(MT):
                    psg = ps_g.tile([P, P], f32, tag="psg", name=f"psg{p_i}")
                    xp = xn_sb[:, :, p_i * P:(p_i + 1) * P]
                    for i in range(ST):
                        nc.tensor.matmul(
                            psg,
                            lhsT=xp[:, i],
                            rhs=xp[:, i],
                            start=(i == 0),
                            stop=(i == ST - 1),
                        )
                    # Diagonal 64x64 blocks only, scaled 1/W_SCALE (ACT).
                    nc.scalar.mul(
                        out=gbd[p_i][0:HD, 0:HD], in_=psg[0:HD, 0:HD],
                        mul=1.0 / W_SCALE,
                    )
                    nc.scalar.mul(
                        out=gbd[p_i][HD:P, HD:P], in_=psg[HD:P, HD:P],
                        mul=1.0 / W_SCALE,
                    )

            def emit_out(p_i):
                for sc in range(SC):
                    pso = ps_o.tile([P, 512], f32, tag="pso", name=f"pso{p_i}_{sc}")
                    nc.tensor.matmul(
                        pso,
                        lhsT=gbd[p_i],
                        rhs=qT_sb[:, p_i, sc * 512:(sc + 1) * 512],
                        start=True,
                        stop=True,
                    )
                    ot = stage.tile([P, 512], f16, tag="ot", name=f"ot{p_i}_{sc}")
                    dr_eng = nc.vector if sc % 2 == 0 else nc.scalar
                    if dr_eng is nc.vector:
                        dr_eng.tensor_copy(out=ot, in_=pso)
                    else:
                        dr_eng.copy(out=ot, in_=pso)
                    nc.gpsimd.dma_start(
                        out=outT_d[p_i * P:(p_i + 1) * P, sc * 512:(sc + 1) * 512],
                        in_=ot,
                    )

            for mt in range(MT):
                psqs = [
                    ps_q.tile([P, 512], f32, tag=f"psq{sc}", name=f"psq{mt}_{sc}")
                    for sc in range(SC)
                ]
                if mt == 0:
                    # PE warmup: self-contained matmuls on the zeroed
                    # scratch; each is a complete start/stop group and the
                    # real kt0 (start=True) overwrites the region.
                    for i in range(N_WARM):
                        nc.tensor.matmul(
                            psqs[0],
                            lhsT=warm_sb[:, 0:P],
                            rhs=warm_sb,
                            start=True,
                            stop=True,
                        )
                for kt in range(KT):
                    for sc in range(SC):
                        nc.tensor.matmul(
                            psqs[sc],
                            lhsT=wT_sb[:, mt, kt],
                            rhs=xT_sb[:, kt, sc * 512:(sc + 1) * 512],
                            start=(kt == 0),
                            stop=(kt == KT - 1),
                        )
                for sc in range(SC):
                    dr_eng = nc.vector if sc < 3 else nc.scalar
                    if dr_eng is nc.vector:
                        dr_eng.tensor_copy(
                            out=qT_sb[:, mt, sc * 512:(sc + 1) * 512], in_=psqs[sc]
                        )
                    else:
                        dr_eng.copy(
                            out=qT_sb[:, mt, sc * 512:(sc + 1) * 512], in_=psqs[sc]
                        )
                if mt == 1:
                    emit_gram()
                if mt == 2:
                    emit_out(0)
                    emit_out(1)
            emit_out(2)
            emit_out(3)
    nc.compile()
    return nc


def _get_nc():
    if "nc" not in _NC_CACHE:
        _NC_CACHE["nc"] = _build_nc()
    return _NC_CACHE["nc"]


def make_in_maps(hidden_states, queries_weight):
    hs = np.ascontiguousarray(np.asarray(hidden_states, dtype=np.float32))
    w = np.ascontiguousarray(np.asarray(queries_weight, dtype=np.float32))
    in_maps = []
    for c in range(N_CORES):
        b, hg = divmod(c, 2)
        xb = hs[b]
        in_maps.append({
            "xT": np.ascontiguousarray(xb.T).astype(np.float16),
            "xn": np.ascontiguousarray(xb[:, hg * MG:(hg + 1) * MG]).astype(
                np.float16
            ),
            "wT": np.ascontiguousarray(
                (w[hg * MG:(hg + 1) * MG, :].T * W_SCALE)
                .reshape(KT, P, MT, P)
                .transpose(2, 1, 0, 3)
                .reshape(MT * P, KT * P)
            ).astype(np.float16),
        })
    return in_maps


def assemble_output(results):
    out = np.empty((B, S, H), dtype=np.float32)
    for c in range(N_CORES):
        b, hg = divmod(c, 2)
        out[b, :, hg * MG:(hg + 1) * MG] = results[c]["outT"].T.astype(np.float32)
    return out


def kernel(hidden_states, queries_weight):
    from concourse.bass_utils import run_bass_kernel_spmd

    in_maps = make_in_maps(hidden_states, queries_weight)
    res = run_bass_kernel_spmd(
        _get_nc(), in_maps, core_ids=list(range(N_CORES))
    ).results
    return assemble_output(res)


if __name__ == "__main__":
    x = np.random.randn(B, S, H).astype(np.float32)
    w = np.random.randn(H, H).astype(np.float32) * 1e-4
    out = kernel(x, w)
    print(out.shape, out.dtype)
